# revision 1
# baseline (speedup 1.0000x reference)
"""CountSketch (scatter variant) as a Bass/Tile kernel for 8 TRN2 NeuronCores.

  out[b, i_hash[j]] += s_hash[j] * x[b, j]
  x: [16384, 8192] f32, i_hash/s_hash: [8192], out: [16384, 2048] f32

Strategy (data-parallel over batch, per sharding hint):
  - Shard batch across 8 cores: each core handles B_CORE=2048 rows.
  - The scatter along the feature axis is a segment-sum over d_in. On TRN2
    the only line-rate "gather/reduce" engine is the PE (matmul reduces over
    the partition dim), so each core's shard is laid out transposed
    [d_in, B_CORE] with d_in on partitions.
  - d_in columns are sorted by hash bucket (host-side, O(d_in) on the tiny
    hash tensors), so the 2048 buckets split into 16 groups of 128 features,
    and each 128-row chunk of the sorted layout touches only 1-2 groups.
  - Per (chunk, group) pair: one [128,128] one-hot +-sign weight matrix W
    (W[j, f] = s_j if i_hash_j == 128g+f else 0), built on device from tiny
    tables; matmul accumulates psum[128 f, B_CORE] over the group's chunks.
  - Host transposes each core's output back and concatenates.

x, W and out are bf16 on device (the harness gate is rel_err < 2e-2;
bf16 rounding contributes ~3e-3): halves x/out HBM traffic and runs the
PE at 1 cycle/row instead of fp32's 4. PSUM accumulation stays fp32.

Scheduling details (all A/B-measured on HW; engine rings are in-order,
so the queue order on each ring must match true dependency order):
  - x/out stay row-major ([d, b]): loads/stores walk DRAM addresses
    densely and sequentially, which measures ~30% faster than a
    partition-contiguous swizzled layout (better HBM locality).
  - All 79 W tiles are built up front on DVE (2.6MB SBUF) so no psum
    cast can head-of-line-block a W build; psum->sbuf casts are
    whole-width per group, alternating ACT/DVE.
  - Loads ride the sync (SP) HWDGE ring, stores the scalar (ACT) ring,
    so stores never block loads. (gpsimd dma_start would be SWDGE:
    ~14us of software descriptor generation.)
The kernel is DMA-bound: ~42MB/core at the ~360-390GB/s per-core share
of chip HBM (all 8 cores streaming) ≈ 107-118us; PE (bf16, ~80us) and
DVE/ACT hide under it.
"""

import ml_dtypes
import numpy as np

import concourse.bass as bass
import concourse.mybir as mybir
import concourse.tile as tile
from concourse import bacc
from concourse.bass_utils import run_bass_kernel_spmd

D_IN = 8192
D_FEATURES = 2048
BATCH = 16384
N_CORES = 8
B_CORE = BATCH // N_CORES  # 2048
P = 128
N_GROUPS = D_FEATURES // P  # 16
N_CHUNKS = D_IN // P  # 64
QN = 512  # matmul moving-operand free-dim limit (one fp32 PSUM bank)
CHUNKS_PER_LOAD = 4  # steady-state chunks per x DMA (2MB bf16)
# (1,) = one extra 1-chunk final load: the drain after the last load is
# 4 matmuls + split casts + a 512KB store (~4.7us) instead of a full
# 4-chunk block's ~9us. Won its quiet paired round; session-best 123486ns.
TAPER = (1,)
X_BUFS = 3  # x-tile pool buffering
STORE_BATCH = 2  # feature groups per output DMA (won 6/6 paired rounds)
# Whole-width nc.any casts measured ~25us faster than per-q casts pinned
# to ACT/DVE: per-q casts head-of-line-block the in-order engine rings
# (queued W builds/stores stall behind a cast waiting on its matmul).
PERQ_CAST = False
CONSTS_RING = "sync"  # "scalar" or "sync": HWDGE ring for const loads
PREBUILD_W = True  # build all W tiles up front (decouple DVE ring)
CAST_MODE = "alt"  # "any" | "alt" (alternate ACT/DVE per group) | "act"
# Pre-issue the first X_BUFS x loads as the sync ring's first DMAs (the
# ring otherwise spends ~2.5us issuing consts first). Needs consts on
# the ACT ring, else const transfers queue behind 6MB of x data.
PREISSUE_X = False
# Drain surgery: the LAST group's cast is split per-q across ACT+DVE
# (parallel ~1.2us instead of one 2.3us cast) and it stores alone
# (512KB) instead of riding a STORE_BATCH pair. Safe with PREBUILD_W:
# there are no W builds left on the DVE ring to head-of-line-block.
LAST_SPLIT = True  # won 4/5 paired rounds; session-best 123976ns

F32 = mybir.dt.float32
BF16 = mybir.dt.bfloat16
I32 = mybir.dt.int32
NP_BF16 = ml_dtypes.bfloat16


def _blocks():
    """Chunk-block schedule: uniform CHUNKS_PER_LOAD blocks with a tapered
    tail so the last loads are small. Returns list of (c0, c1)."""
    taper = [t for t in TAPER if t < CHUNKS_PER_LOAD]
    tail = sum(taper)
    n_uniform = (N_CHUNKS - tail) // CHUNKS_PER_LOAD
    sizes = [CHUNKS_PER_LOAD] * n_uniform
    rem = N_CHUNKS - tail - n_uniform * CHUNKS_PER_LOAD
    if rem:
        sizes.append(rem)
    sizes += taper
    out, c = [], 0
    for s in sizes:
        out.append((c, c + s))
        c += s
    assert c == N_CHUNKS
    return out


def _plan(i_hash: np.ndarray, s_hash: np.ndarray):
    """Host-side schedule from the tiny hash tensors."""
    i_hash = np.asarray(i_hash).astype(np.int64)
    s_hash = np.asarray(s_hash).astype(np.float32)
    perm = np.argsort(i_hash, kind="stable")
    b_sorted = i_hash[perm]
    g_sorted = b_sorted // P
    f_sorted = (b_sorted % P).astype(np.float32)
    s_sorted = s_hash[perm]

    pairs = []  # (chunk, group, f_local col f32[128], sign col f32[128])
    for c in range(N_CHUNKS):
        sl = slice(P * c, P * (c + 1))
        gs = g_sorted[sl]
        for g in sorted(set(gs.tolist())):
            m = gs == g
            pairs.append(
                (
                    c,
                    int(g),
                    np.where(m, f_sorted[sl], -1.0).astype(np.float32),
                    np.where(m, s_sorted[sl], 0.0).astype(np.float32),
                )
            )

    first_pair = {}
    last_pair = {}
    for idx, (c, g, _, _) in enumerate(pairs):
        first_pair.setdefault(g, idx)
        last_pair[g] = idx

    ftab = np.stack([p[2] for p in pairs], axis=1)  # [P, n_pairs]
    stab = np.stack([p[3] for p in pairs], axis=1)  # [P, n_pairs]
    meta = [(c, g) for (c, g, _, _) in pairs]
    return perm, meta, ftab, stab, first_pair, last_pair


def _build_nc(meta, first_pair, last_pair, n_pairs, b_core=B_CORE, reps=1):
    nc = bacc.Bacc(None, target_bir_lowering=False)
    x_t = nc.dram_tensor("x_t", [D_IN, b_core], BF16, kind="ExternalInput")
    # ftab | stab | iota packed into one tensor -> one const DMA (one
    # 625ns ring-issue slot ahead of the x loads instead of three)
    ctab_d = nc.dram_tensor(
        "ctab", [P, 2 * n_pairs + P], F32, kind="ExternalInput"
    )
    out_t = nc.dram_tensor("out_t", [D_FEATURES, b_core], BF16, kind="ExternalOutput")

    qn = min(QN, b_core)
    n_q = b_core // qn
    blocks = _blocks()
    blk_of_chunk = {}
    for bi, (c0, c1) in enumerate(blocks):
        for c in range(c0, c1):
            blk_of_chunk[c] = bi
    from contextlib import ExitStack

    with tile.TileContext(nc) as tc, ExitStack() as ctx:
        consts = ctx.enter_context(tc.tile_pool(name="consts", bufs=1))
        xpool = ctx.enter_context(tc.tile_pool(name="x", bufs=X_BUFS))
        wpool = ctx.enter_context(tc.tile_pool(name="w", bufs=4))
        wbig = ctx.enter_context(tc.tile_pool(name="wbig", bufs=1))
        psum = ctx.enter_context(
            tc.tile_pool(name="psum", bufs=2, space=bass.MemorySpace.PSUM)
        )
        opool = ctx.enter_context(tc.tile_pool(name="o", bufs=2))

        x_tiles = {}  # (rep, block index) -> tile

        def get_x_tile(c, rep=0):
            bi = blk_of_chunk[c]
            c0, c1 = blocks[bi]
            key = (rep, bi)
            if key not in x_tiles:
                xt = xpool.tile([P, CHUNKS_PER_LOAD, b_core], BF16)
                nc.sync.dma_start(
                    xt[:, : c1 - c0, :],
                    x_t[c0 * P : c1 * P, :].rearrange("(c p) n -> p c n", p=P),
                )
                x_tiles[key] = xt
            return x_tiles[key], c - c0

        if PREISSUE_X:
            for bi in range(min(X_BUFS, len(blocks))):
                get_x_tile(blocks[bi][0], 0)

        # consts on a HWDGE ring ("scalar" keeps them off the sync ring so
        # the sync ring's first DMA is x block 0; gpsimd dma_start would
        # be SWDGE: ~14us of software descriptor generation delaying
        # every W build)
        use_scalar = CONSTS_RING == "scalar" or PREISSUE_X
        const_ring = nc.scalar if use_scalar else nc.sync
        ctab_sb = consts.tile([P, 2 * n_pairs + P], F32)
        const_ring.dma_start(ctab_sb[:], ctab_d[:])
        ftab_sb = ctab_sb[:, 0:n_pairs]
        stab_sb = ctab_sb[:, n_pairs : 2 * n_pairs]
        iota_sb = ctab_sb[:, 2 * n_pairs : 2 * n_pairs + P]

        def build_w(out_ap, pi):
            # W[j, f] = (iota_f == f_local_j) * s_j ([P, P] one-hot, bf16)
            # in one fused DVE op: (iota is_equal ftab_j) mult s_j
            nc.vector.tensor_scalar(
                out=out_ap,
                in0=iota_sb[:],
                scalar1=ftab_sb[:, pi : pi + 1],
                scalar2=stab_sb[:, pi : pi + 1],
                op0=mybir.AluOpType.is_equal,
                op1=mybir.AluOpType.mult,
            )

        # store batching plan: contiguous group runs of STORE_BATCH; with
        # LAST_SPLIT the final two groups store alone so the drain's last
        # store is 512KB, not a 1MB pair
        singles = 2 if (LAST_SPLIT and N_GROUPS >= 2) else 0
        sbatches = [
            list(range(i, min(i + STORE_BATCH, N_GROUPS - singles)))
            for i in range(0, N_GROUPS - singles, STORE_BATCH)
        ] + [[g] for g in range(N_GROUPS - singles, N_GROUPS)]
        g_to_batch = {}
        for b_i, gs in enumerate(sbatches):
            for k, g in enumerate(gs):
                g_to_batch[g] = (b_i, k, len(gs))

        for rep in range(reps):
            wtab = None
            if PREBUILD_W:
                # all W tiles up front: the DVE ring then has no W build
                # that could queue behind a cast waiting on a matmul
                wtab = wbig.tile([P, n_pairs, P], BF16, tag="wtab")
                for pi in range(n_pairs):
                    build_w(wtab[:, pi, :], pi)

            cur_psum = None
            for pi, (c, g) in enumerate(meta):
                xt, ci = get_x_tile(c, rep)
                if PREBUILD_W:
                    w = wtab[:, pi, :]
                else:
                    wt = wpool.tile([P, P], BF16, tag="w")
                    build_w(wt[:], pi)
                    w = wt[:]

                b_i, k, bsz = g_to_batch[g]
                last_g_split = LAST_SPLIT and g == N_GROUPS - 1
                if pi == first_pair[g]:
                    cur_psum = psum.tile([P, b_core], F32)
                if pi == last_pair[g] and k == 0:
                    cur_ot = opool.tile([P, STORE_BATCH, b_core], BF16)
                for q in range(n_q):
                    nc.tensor.matmul(
                        cur_psum[:, q * qn : (q + 1) * qn],
                        lhsT=w[:],
                        rhs=xt[:, ci, q * qn : (q + 1) * qn],
                        start=(pi == first_pair[g]),
                        stop=(pi == last_pair[g]),
                    )
                    if pi == last_pair[g] and last_g_split:
                        # final group: per-q casts split across ACT+DVE run
                        # in parallel behind their stop-matmuls (~1.2us
                        # total vs one 2.3us whole-width cast)
                        dst = cur_ot[:, k, q * qn : (q + 1) * qn]
                        src = cur_psum[:, q * qn : (q + 1) * qn]
                        if q < n_q // 2:
                            nc.scalar.copy(dst, src)
                        else:
                            nc.vector.tensor_copy(dst, src)
                if pi == last_pair[g] and not last_g_split:
                    dst, src = cur_ot[:, k, :], cur_psum[:]
                    if CAST_MODE == "any":
                        nc.any.tensor_copy(dst, src)
                    elif CAST_MODE == "act" or (CAST_MODE == "alt" and b_i % 2 == 0):
                        nc.scalar.copy(dst, src)
                    else:
                        nc.vector.tensor_copy(dst, src)
                if pi == last_pair[g] and k == bsz - 1:
                    g0 = sbatches[b_i][0]
                    # scalar (ACT) HWDGE ring: keeps stores off the sync
                    # ring so they can't head-of-line-block x loads
                    nc.scalar.dma_start(
                        out_t[g0 * P : (g0 + bsz) * P, :].rearrange(
                            "(c p) n -> p c n", p=P
                        ),
                        cur_ot[:, :bsz, :],
                    )

            # Groups with no hashed columns (possible in principle): zero-fill.
            for g in range(N_GROUPS):
                if g not in first_pair:
                    ot = opool.tile([P, b_core], BF16, tag="zfill")
                    nc.vector.memset(ot[:], 0.0)
                    nc.scalar.dma_start(out_t[g * P : (g + 1) * P, :], ot[:])

    nc.finalize()
    return nc


def _pack_consts(ftab, stab):
    """ftab | stab | iota -> [P, 2*n_pairs + P] f32 (single const DMA)."""
    iota = np.broadcast_to(np.arange(P, dtype=np.float32), (P, P))
    return np.ascontiguousarray(
        np.concatenate([ftab, stab, iota], axis=1).astype(np.float32)
    )


def _prepare(x, s_hash, i_hash):
    """Build the per-core input maps and the compiled Bass program."""
    perm, meta, ftab, stab, first_pair, last_pair = _plan(i_hash, s_hash)
    nc = _build_nc(meta, first_pair, last_pair, len(meta))

    ctab = _pack_consts(ftab, stab)

    x = np.asarray(x, dtype=np.float32)
    in_maps = []
    for core in range(N_CORES):
        sh = x[core * B_CORE : (core + 1) * B_CORE]  # [B_CORE, D_IN]
        x_dev = np.ascontiguousarray(sh.T[perm].astype(NP_BF16))  # sorted
        in_maps.append({"x_t": x_dev, "ctab": ctab})
    return nc, in_maps


def _run(x, s_hash, i_hash, trace=False, **kw):
    nc, in_maps = _prepare(x, s_hash, i_hash)
    res = run_bass_kernel_spmd(nc, in_maps, list(range(N_CORES)), trace=trace, **kw)
    out = np.concatenate(
        [
            np.asarray(res.results[i]["out_t"]).astype(np.float32).T
            for i in range(N_CORES)
        ],
        axis=0,
    )
    return out, res


def kernel(x, s_hash, i_hash):
    out, _ = _run(x, s_hash, i_hash)
    return out

